# revision 3
# baseline (speedup 1.0000x reference)
"""AlphaNet_v1 Trainium2 kernel — 8-core data-parallel Bass implementation.

Structure (per core, shard = 16384 rows of xb):
  NEFF-A: one pass over the shard. Per 128-row chunk: compute the raw
      (scale-folded) AlphaNet features F [128, 702] in fp16, accumulate
      per-column sum / sum-of-squares via PE column-sum matmuls, PE-transpose
      F and spill F^T to DRAM.
  Host: all-reduce the per-column sums across cores (numpy), compute the
      BatchNorm affine per feature column, and fold it into the MLP weights
      (BatchNorm is an affine a*x+b per tensor; max/min over windows commute
      because a>0 with gamma=1; feature scale factors are absorbed exactly by
      adjusting BN's epsilon per group).
  NEFF-B: stream F^T back, 6 accumulated matmuls against the folded W1',
      fused bias+relu on the scalar engine, final matmul against W2', output.

Stored feature scales (ref = s * stored): corr 1, cov(S=window sum of c_i c_j)
1/9, sd(=sqrt(S_ii/10)) sqrt(10/9), zs(=m/(sd+K*eps)) K=sqrt(9/10), q(=xl/xf,
shift by -1 is BN-invariant) 1, decay(=sum x*d) 1/55, mean 1; ubar stores the
window SUM (handled in fold).
"""

import contextlib
import numpy as np

import bass_rust
import concourse.bass as bass
import concourse.mybir as mybir
import concourse.tile as tile
from concourse.bass_utils import run_bass_kernel_spmd

F32 = mybir.dt.float32
FP16 = mybir.dt.float16
ALU = mybir.AluOpType
AF = mybir.ActivationFunctionType
AX = mybir.AxisListType

NCORES = 8
B_TOTAL = 131072
ROWS = B_TOTAL // NCORES          # 16384 rows per core
CHUNK = 128
NCHUNKS = ROWS // CHUNK           # 128
NF, NW, ND = 9, 3, 10             # features, windows, days-per-window
NPAIR = 36
NFEAT = 117                       # 36+36+9*5
FCOLS = 702                       # 117*3 + 3*117
FPAD = 768
EPS_BN, EPS = 1e-5, 1e-8
KSTD = float(np.sqrt(0.9))

I_IDX, J_IDX = np.triu_indices(NF, k=1)
GROUP_SIZES = [36, 36, 9, 9, 9, 9, 9]
# ref = s * stored, per xcat group [corr, cov, sd, zs, q, decay, m]
S_T = [1.0, 1.0 / 9.0, float(np.sqrt(10.0 / 9.0)), KSTD, 1.0, 1.0 / 55.0, 1.0]


# ---------------------------------------------------------------------------
# toolchain workaround: this walrus build allows only ONE semaphore wait per
# instruction; Tile sometimes attaches more. Hoist extras onto standalone
# Drain instructions inserted before the offender on the same engine.
_wsplit_n = [0]


def split_multi_waits(nc):
    for fn in nc.m.functions:
        for bb in fn.blocks:
            new_list = []
            for ins in bb.instructions:
                si = ins.sync_info
                waits = list(si.on_wait) if (si is not None and si.on_wait) else []
                if len(waits) > 1:
                    for w in waits[:-1]:
                        _wsplit_n[0] += 1
                        d = bass_rust.InstDrain(
                            name=f"wsplit-{_wsplit_n[0]}", ins=[], outs=[]
                        )
                        d.engine = ins.engine
                        d.sync_info = mybir.SyncInfo(on_wait=[w], on_update=[])
                        new_list.append(d)
                    si.on_wait = [waits[-1]]
                new_list.append(ins)
            bb.instructions[:] = new_list


# ---------------------------------------------------------------------------
def build_neff_a(n_chunks=NCHUNKS):
    nc = bass.Bass()
    x_ext = nc.declare_dram_parameter("x", [n_chunks * CHUNK, 270], F32, isOutput=False)
    w_ext = nc.declare_dram_parameter("wday", [128, 30], F32, isOutput=False)
    id_ext = nc.declare_dram_parameter("ident", [128, 128], FP16, isOutput=False)
    sums_ext = nc.declare_dram_parameter("sums", [1, 2 * FCOLS], F32, isOutput=True)
    ft_ext = nc.declare_dram_parameter("ft", [n_chunks, FPAD, CHUNK], FP16, isOutput=True)

    ctx = contextlib.ExitStack()
    with ctx:
        ctx.enter_context(nc.allow_low_precision("fp16 feature storage by design"))
        tc = ctx.enter_context(tile.TileContext(nc))
        const = ctx.enter_context(tc.tile_pool(name="const", bufs=1))
        work = ctx.enter_context(tc.tile_pool(name="work", bufs=2))
        fpool = ctx.enter_context(tc.tile_pool(name="fpool", bufs=2))
        acc = ctx.enter_context(tc.tile_pool(name="acc", bufs=1, space="PSUM"))
        tp = ctx.enter_context(tc.tile_pool(name="tp", bufs=3, space="PSUM"))

        w32 = const.tile([128, 30], F32, tag="w32")
        ident = const.tile([128, 128], FP16, tag="ident")
        ones16 = const.tile([128, 1], FP16, tag="ones16")
        ones32 = const.tile([128, 1], F32, tag="ones32")
        nc.sync.dma_start(w32[:], w_ext[:])
        nc.sync.dma_start(ident[:], id_ext[:])
        nc.vector.memset(ones16[:], 1.0)
        nc.vector.memset(ones32[:], 1.0)

        # persistent PSUM accumulators for per-column sums
        sF_a = acc.tile([1, 512], F32, tag="sF_a")
        sF_b = acc.tile([1, FCOLS - 512], F32, tag="sF_b")
        sQ_a = acc.tile([1, 512], F32, tag="sQ_a")
        sQ_b = acc.tile([1, FCOLS - 512], F32, tag="sQ_b")

        for c in range(n_chunks):
            x32 = work.tile([128, 270], F32, tag="x32")
            nc.sync.dma_start(x32[:], x_ext[c * CHUNK:(c + 1) * CHUNK, :])
            xv = x32[:].rearrange("p (f w d) -> p f w d", f=NF, w=NW, d=ND)

            # window means (exact, f32)
            msum = work.tile([128, 27], F32, tag="msum")
            nc.vector.tensor_reduce(out=msum[:], in_=xv, axis=AX.X, op=ALU.add)
            m32 = work.tile([128, 27], F32, tag="m32")
            nc.vector.tensor_scalar_mul(m32[:], msum[:], 0.1)
            mv = m32[:].rearrange("p (f w) -> p f w", f=NF, w=NW)

            # centered values, fp16 (proportional rounding)
            c16 = work.tile([128, 270], F32, tag="c16_f32")  # keep f32: products need precision at tiny c
            nc.gpsimd.tensor_tensor(
                out=c16[:].rearrange("p (f w d) -> p f w d", f=NF, w=NW, d=ND),
                in0=xv,
                in1=mv.unsqueeze(3).broadcast_to([128, NF, NW, ND]),
                op=ALU.subtract,
            )
            cv = c16[:].rearrange("p (f w d) -> p f w d", f=NF, w=NW, d=ND)

            # diag products c*c -> fp16
            cd16 = work.tile([128, 270], FP16, tag="cd16")
            nc.gpsimd.tensor_tensor(out=cd16[:], in0=c16[:], in1=c16[:], op=ALU.mult)

            # off-diagonal pair products -> fp16 [128, 36*30]
            p16 = work.tile([128, NPAIR * 30], FP16, tag="p16")
            pv = p16[:].rearrange("p (k w d) -> p k w d", k=NPAIR, w=NW, d=ND)
            base = 0
            for i in range(NF - 1):
                nj = NF - 1 - i
                nc.vector.tensor_tensor(
                    out=pv[:, base:base + nj],
                    in0=cv[:, i:i + 1].broadcast_to([128, nj, NW, ND]),
                    in1=cv[:, i + 1:],
                    op=ALU.mult,
                )
                base += nj

            # window sums
            varS = work.tile([128, 27], F32, tag="varS")
            nc.vector.tensor_reduce(
                out=varS[:],
                in_=cd16[:].rearrange("p (f w d) -> p f w d", f=NF, w=NW, d=ND),
                axis=AX.X, op=ALU.add,
            )
            S32 = work.tile([128, 108], F32, tag="S32")
            nc.vector.tensor_reduce(
                out=S32[:],
                in_=pv,
                axis=AX.X, op=ALU.add,
            )

            # F assembly
            F = fpool.tile([128, FPAD], FP16, tag="F")
            nc.gpsimd.memset(F[:, FCOLS:FPAD], 0.0)

            # sd = sqrt(varS/10)  (= K * std)
            sd32 = work.tile([128, 27], F32, tag="sd32")
            nc.scalar.activation(out=sd32[:], in_=varS[:], func=AF.Sqrt, scale=0.1)
            nc.scalar.copy(out=F[:, 216:243], in_=sd32[:])
            nc.scalar.copy(out=F[:, 324:351], in_=m32[:])
            nc.scalar.copy(out=F[:, 108:216], in_=S32[:])

            # corr = S / (10*sd_i*sd_j + 9e-8)
            den = work.tile([128, 108], F32, tag="den")
            dnv = den[:].rearrange("p (k w) -> p k w", k=NPAIR, w=NW)
            sdv = sd32[:].rearrange("p (f w) -> p f w", f=NF, w=NW)
            base = 0
            for i in range(NF - 1):
                nj = NF - 1 - i
                nc.vector.tensor_tensor(
                    out=dnv[:, base:base + nj],
                    in0=sdv[:, i:i + 1].broadcast_to([128, nj, NW]),
                    in1=sdv[:, i + 1:],
                    op=ALU.mult,
                )
                base += nj
            nc.vector.tensor_scalar(
                out=den[:], in0=den[:], scalar1=10.0, scalar2=9.0 * EPS,
                op0=ALU.mult, op1=ALU.add,
            )
            rec = work.tile([128, 108], F32, tag="rec")
            nc.vector.reciprocal(out=rec[:], in_=den[:])
            nc.vector.tensor_tensor(out=F[:, 0:108], in0=S32[:], in1=rec[:], op=ALU.mult)

            # zscore' = m / (sd + K*eps)
            zden = work.tile([128, 27], F32, tag="zden")
            nc.vector.tensor_scalar_add(zden[:], sd32[:], KSTD * EPS)
            zrec = work.tile([128, 27], F32, tag="zrec")
            nc.vector.reciprocal(out=zrec[:], in_=zden[:])
            nc.vector.tensor_tensor(out=F[:, 243:270], in0=m32[:], in1=zrec[:], op=ALU.mult)

            # q = x_last / x_first
            qrec = work.tile([128, 27], F32, tag="qrec")
            nc.vector.reciprocal(out=qrec[:], in_=xv[:, :, :, 0])
            nc.vector.tensor_tensor(out=F[:, 270:297], in0=xv[:, :, :, ND - 1], in1=qrec[:], op=ALU.mult)

            # decay' = sum_d x*d
            dw16 = work.tile([128, 270], FP16, tag="dw16")
            nc.gpsimd.tensor_tensor(
                out=dw16[:].rearrange("p (f w d) -> p f w d", f=NF, w=NW, d=ND),
                in0=xv,
                in1=w32[:].rearrange("p (w d) -> p w d", w=NW, d=ND)
                    .unsqueeze(1).broadcast_to([128, NF, NW, ND]),
                op=ALU.mult,
            )
            nc.vector.tensor_reduce(
                out=F[:, 297:324],
                in_=dw16[:].rearrange("p (f w d) -> p f w d", f=NF, w=NW, d=ND),
                axis=AX.X, op=ALU.add,
            )

            # window aggregates over the 351 xcat columns
            fu = F[:, 0:351].rearrange("p (f w) -> p f w", f=NFEAT, w=NW)
            tmpa = work.tile([128, NFEAT], FP16, tag="tmpa")
            nc.vector.tensor_tensor(out=tmpa[:], in0=fu[:, :, 0], in1=fu[:, :, 1], op=ALU.add)
            nc.vector.tensor_tensor(out=F[:, 351:468], in0=tmpa[:], in1=fu[:, :, 2], op=ALU.add)
            tmpb = work.tile([128, NFEAT], FP16, tag="tmpb")
            nc.vector.tensor_tensor(out=tmpb[:], in0=fu[:, :, 0], in1=fu[:, :, 1], op=ALU.max)
            nc.vector.tensor_tensor(out=F[:, 468:585], in0=tmpb[:], in1=fu[:, :, 2], op=ALU.max)
            tmpc = work.tile([128, NFEAT], FP16, tag="tmpc")
            nc.vector.tensor_tensor(out=tmpc[:], in0=fu[:, :, 0], in1=fu[:, :, 1], op=ALU.min)
            nc.vector.tensor_tensor(out=F[:, 585:702], in0=tmpc[:], in1=fu[:, :, 2], op=ALU.min)

            # stats: column sums of F and F^2
            F2 = fpool.tile([128, FCOLS], F32, tag="F2")
            nc.scalar.activation(out=F2[:], in_=F[:, 0:FCOLS], func=AF.Square)
            first, last = (c == 0), (c == n_chunks - 1)
            nc.tensor.matmul(out=sF_a[:], lhsT=ones16[:], rhs=F[:, 0:512],
                             start=first, stop=last)
            nc.tensor.matmul(out=sF_b[:], lhsT=ones16[:], rhs=F[:, 512:FCOLS],
                             start=first, stop=last)
            nc.tensor.matmul(out=sQ_a[:], lhsT=ones32[:], rhs=F2[:, 0:512],
                             start=first, stop=last)
            nc.tensor.matmul(out=sQ_b[:], lhsT=ones32[:], rhs=F2[:, 512:FCOLS],
                             start=first, stop=last)

            # transpose F -> ft (6 pieces of 128 cols)
            ftile = fpool.tile([128, 6 * 128], FP16, tag="ftile")
            for p in range(6):
                pt = tp.tile([128, 128], FP16, tag="pt")
                nc.tensor.transpose(out=pt[:], in_=F[:, p * 128:(p + 1) * 128],
                                    identity=ident[:])
                eng = nc.scalar if p % 2 == 0 else nc.vector
                if eng is nc.scalar:
                    nc.scalar.copy(out=ftile[:, p * 128:(p + 1) * 128], in_=pt[:])
                else:
                    nc.vector.tensor_copy(ftile[:, p * 128:(p + 1) * 128], pt[:])
            nc.sync.dma_start(
                ft_ext[c].rearrange("(k p) r -> p k r", k=6, p=128),
                ftile[:].rearrange("p (k r) -> p k r", k=6, r=128),
            )

        # evacuate sums
        sums_sb = const.tile([1, 2 * FCOLS], F32, tag="sums_sb")
        nc.scalar.copy(out=sums_sb[:, 0:512], in_=sF_a[:])
        nc.scalar.copy(out=sums_sb[:, 512:FCOLS], in_=sF_b[:])
        nc.scalar.copy(out=sums_sb[:, FCOLS:FCOLS + 512], in_=sQ_a[:])
        nc.scalar.copy(out=sums_sb[:, FCOLS + 512:2 * FCOLS], in_=sQ_b[:])
        nc.sync.dma_start(sums_ext[:], sums_sb[:])

    split_multi_waits(nc)
    return nc


# ---------------------------------------------------------------------------
def build_neff_b(n_chunks=NCHUNKS):
    nc = bass.Bass()
    ft_ext = nc.declare_dram_parameter("ft", [n_chunks, FPAD, CHUNK], FP16, isOutput=False)
    w1_ext = nc.declare_dram_parameter("w1t", [FPAD, 30], FP16, isOutput=False)
    b1_ext = nc.declare_dram_parameter("b1p", [30, 1], F32, isOutput=False)
    w2_ext = nc.declare_dram_parameter("w2p", [30, 1], FP16, isOutput=False)
    bo_ext = nc.declare_dram_parameter("boutp", [1, 1], F32, isOutput=False)
    out_ext = nc.declare_dram_parameter("out", [1, n_chunks * CHUNK], F32, isOutput=True)

    ctx = contextlib.ExitStack()
    with ctx:
        ctx.enter_context(nc.allow_low_precision("fp16 mlp by design"))
        tc = ctx.enter_context(tile.TileContext(nc))
        const = ctx.enter_context(tc.tile_pool(name="const", bufs=1))
        work = ctx.enter_context(tc.tile_pool(name="work", bufs=3))
        ps = ctx.enter_context(tc.tile_pool(name="ps", bufs=2, space="PSUM"))

        w1b = const.tile([128, 6 * 30], FP16, tag="w1b")
        nc.sync.dma_start(
            w1b[:].rearrange("p (k m) -> p k m", k=6, m=30),
            w1_ext[:].rearrange("(k p) m -> p k m", k=6, p=128),
        )
        b1b = const.tile([30, 1], F32, tag="b1b")
        nc.sync.dma_start(b1b[:], b1_ext[:])
        w2b = const.tile([30, 1], FP16, tag="w2b")
        nc.sync.dma_start(w2b[:], w2_ext[:])
        bob = const.tile([1, 1], F32, tag="bob")
        nc.sync.dma_start(bob[:], bo_ext[:])
        out_sb = const.tile([1, n_chunks * CHUNK], F32, tag="out_sb")

        for c in range(n_chunks):
            ftb = work.tile([128, 6 * 128], FP16, tag="ftb")
            nc.sync.dma_start(
                ftb[:].rearrange("p (k r) -> p k r", k=6, r=128),
                ft_ext[c].rearrange("(k p) r -> p k r", k=6, p=128),
            )
            h_ps = ps.tile([30, 128], F32, tag="h_ps")
            for p in range(6):
                nc.tensor.matmul(
                    out=h_ps[:],
                    lhsT=w1b[:, p * 30:(p + 1) * 30],
                    rhs=ftb[:, p * 128:(p + 1) * 128],
                    start=(p == 0), stop=(p == 5),
                )
            h16 = work.tile([30, 128], FP16, tag="h16")
            nc.scalar.activation(out=h16[:], in_=h_ps[:], func=AF.Relu,
                                 bias=b1b[:], scale=1.0)
            o_ps = ps.tile([1, 128], F32, tag="o_ps")
            nc.tensor.matmul(out=o_ps[:], lhsT=w2b[:], rhs=h16[:],
                             start=True, stop=True)
            nc.scalar.activation(out=out_sb[:, c * CHUNK:(c + 1) * CHUNK],
                                 in_=o_ps[:], func=AF.Identity,
                                 bias=bob[:], scale=1.0)

        nc.sync.dma_start(out_ext[:], out_sb[:])

    split_multi_waits(nc)
    return nc


# ---------------------------------------------------------------------------
def fold_weights(sums, n_rows, gamma, beta, W1, b1, W2, b2, w_out, b_out):
    """sums: [2, 702] float64 global column sums / sumsqs."""
    s1, s2 = sums[0], sums[1]
    alpha = np.zeros(FCOLS)
    bet = np.zeros(FCOLS)
    mu_col = s1 / n_rows
    for g, sz in enumerate(GROUP_SIZES):
        f0 = sum(GROUP_SIZES[:g])
        cols = slice(f0 * 3, (f0 + sz) * 3)
        cnt = n_rows * sz * 3
        mean = s1[cols].sum() / cnt
        var = s2[cols].sum() / cnt - mean ** 2
        a = gamma / np.sqrt(var + EPS_BN / S_T[g] ** 2)
        alpha[cols] = a
        bet[cols] = beta - a * mean
    a_f = alpha[np.arange(NFEAT) * 3]
    b_f = bet[np.arange(NFEAT) * 3]
    for blk, scale in ((0, 1.0 / 3.0), (1, 1.0), (2, 1.0)):
        cols = slice(351 + blk * NFEAT, 351 + (blk + 1) * NFEAT)
        e1 = s1[cols] / n_rows
        e2 = s2[cols] / n_rows
        mean_all = (a_f * scale * e1 + b_f).mean()
        ex2_all = ((a_f * scale) ** 2 * e2 + 2 * a_f * scale * b_f * e1 + b_f ** 2).mean()
        var_all = ex2_all - mean_all ** 2
        a2 = gamma / np.sqrt(var_all + EPS_BN)
        b2_ = beta - a2 * mean_all
        alpha[cols] = a2 * a_f * scale
        bet[cols] = a2 * b_f + b2_
    W1p = W1 * alpha[None, :]
    b1p = b1 + W1 @ bet
    w1t = np.zeros((FPAD, 30), np.float16)
    w1t[:FCOLS, :] = W1p.T.astype(np.float16)
    b1p = b1p.reshape(30, 1).astype(np.float32)
    w2p = (W2[0] * float(w_out[0])).reshape(30, 1).astype(np.float16)
    boutp = np.array([[float(b2[0]) * float(w_out[0]) + float(b_out[0])]], np.float32)
    return w1t, b1p, w2p, boutp


# ---------------------------------------------------------------------------
_CACHE = {}


def _get_neffs():
    if "a" not in _CACHE:
        _CACHE["a"] = build_neff_a()
        _CACHE["b"] = build_neff_b()
    return _CACHE["a"], _CACHE["b"]


def _wday_tile():
    w = np.tile(np.arange(1, ND + 1, dtype=np.float32), NW)  # [30]
    return np.tile(w[None, :], (128, 1)).copy()


def kernel(xb, gamma, beta, W1, b1, W2, b2, w_out, b_out):
    xb = np.ascontiguousarray(np.asarray(xb, dtype=np.float32))
    x_flat = xb.reshape(B_TOTAL, 270)
    shards = [np.ascontiguousarray(x_flat[i * ROWS:(i + 1) * ROWS]) for i in range(NCORES)]

    nc_a, nc_b = _get_neffs()
    wday = _wday_tile()
    ident = np.eye(128, dtype=np.float16)

    in_maps_a = [{"x": shards[i], "wday": wday, "ident": ident} for i in range(NCORES)]
    res_a = run_bass_kernel_spmd(nc_a, in_maps_a, core_ids=list(range(NCORES)))

    sums = np.zeros((2, FCOLS), np.float64)
    fts = []
    for i in range(NCORES):
        s = res_a.results[i]["sums"].reshape(2, FCOLS).astype(np.float64)
        sums += s
        fts.append(res_a.results[i]["ft"])

    w1t, b1p, w2p, boutp = fold_weights(
        sums, B_TOTAL,
        float(np.asarray(gamma).reshape(-1)[0]), float(np.asarray(beta).reshape(-1)[0]),
        np.asarray(W1, np.float64), np.asarray(b1, np.float64),
        np.asarray(W2, np.float64), np.asarray(b2, np.float64),
        np.asarray(w_out, np.float64), np.asarray(b_out, np.float64),
    )

    in_maps_b = [
        {"ft": fts[i], "w1t": w1t, "b1p": b1p, "w2p": w2p, "boutp": boutp}
        for i in range(NCORES)
    ]
    res_b = run_bass_kernel_spmd(nc_b, in_maps_b, core_ids=list(range(NCORES)))

    out = np.concatenate([res_b.results[i]["out"].reshape(-1) for i in range(NCORES)])
    return out.astype(np.float32)


# revision 6
# speedup vs baseline: 1.1462x; 1.1462x over previous
"""AlphaNet_v1 Trainium2 kernel — 8-core data-parallel Bass implementation.

Structure (per core, shard = 16384 rows of xb):
  NEFF-A: one pass over the shard, G=4 chunks of 128 rows per iteration.
      Compute the raw (scale-folded) AlphaNet features F [128, G*702] fp16,
      accumulate per-column sum / sum-of-squares via PE column-sum matmuls,
      PE-transpose F and spill F^T to DRAM.
  Host: reduce the column sums across cores (numpy), build the BatchNorm
      affine per feature column, fold into the MLP weights (BN is affine
      a*x+b per tensor; max/min over windows commute since a>0 for gamma=1;
      stored-feature scale factors are absorbed exactly by scaling BN's
      epsilon per group).
  NEFF-B: stream F^T back, accumulated matmuls against folded W1', fused
      bias+relu on ScalarE, final matmul against W2', output.

Stored feature scales (ref = s * stored): corr 1, cov(S = window sum of
c_i*c_j) 1/9, sd(=sqrt(S_ii/10)) sqrt(10/9), zs(=m/(sd+K*eps)) K=sqrt(9/10),
q(=xl/xf; the -1 shift is BN-invariant) 1, decay(=sum x*d) 1/55, mean 1;
ubar stores the window SUM (handled in fold).
"""

import contextlib
import numpy as np

import bass_rust
import concourse.bass as bass
import concourse.mybir as mybir
import concourse.tile as tile
from concourse.bass_utils import run_bass_kernel_spmd

F32 = mybir.dt.float32
FP16 = mybir.dt.float16
BF16 = mybir.dt.bfloat16
ALU = mybir.AluOpType
AF = mybir.ActivationFunctionType
AX = mybir.AxisListType

NCORES = 8
B_TOTAL = 131072
ROWS = B_TOTAL // NCORES          # 16384 rows per core
CHUNK = 128
NCHUNKS = ROWS // CHUNK           # 128
GA = 4                            # chunks per iteration, NEFF-A
GB = 4                            # chunks per iteration, NEFF-B
NF, NW, ND = 9, 3, 10
NPAIR = 36
NFEAT = 117
FCOLS = 702
FPAD = 768
EPS_BN, EPS = 1e-5, 1e-8
KSTD = float(np.sqrt(0.9))

I_IDX, J_IDX = np.triu_indices(NF, k=1)
GROUP_SIZES = [36, 36, 9, 9, 9, 9, 9]
# ref = s * stored, per xcat group [corr, cov, sd, zs, q, decay, m]
S_T = [1.0, 1.0 / 9.0, float(np.sqrt(10.0 / 9.0)), KSTD, 1.0, 1.0 / 55.0, 1.0]


# ---------------------------------------------------------------------------
# toolchain workaround: this walrus build allows only ONE semaphore wait per
# instruction; Tile sometimes attaches more. Hoist extras onto standalone
# Drain instructions inserted before the offender on the same engine.
_wsplit_n = [0]


def split_multi_waits(nc):
    for fn in nc.m.functions:
        for bb in fn.blocks:
            new_list = []
            for ins in bb.instructions:
                si = ins.sync_info
                waits = list(si.on_wait) if (si is not None and si.on_wait) else []
                if len(waits) > 1:
                    for w in waits[:-1]:
                        _wsplit_n[0] += 1
                        d = bass_rust.InstDrain(
                            name=f"wsplit-{_wsplit_n[0]}", ins=[], outs=[]
                        )
                        d.engine = ins.engine
                        d.sync_info = mybir.SyncInfo(on_wait=[w], on_update=[])
                        new_list.append(d)
                    si.on_wait = [waits[-1]]
                new_list.append(ins)
            bb.instructions[:] = new_list


# ---------------------------------------------------------------------------
def build_neff_a(n_chunks=NCHUNKS, g=GA):
    assert n_chunks % g == 0
    ngroups = n_chunks // g
    nc = bass.Bass()
    x_ext = nc.declare_dram_parameter("x", [n_chunks * CHUNK, 270], F32, isOutput=False)
    w_ext = nc.declare_dram_parameter("wday", [128, 30], F32, isOutput=False)
    id_ext = nc.declare_dram_parameter("ident", [128, 128], FP16, isOutput=False)
    sums_ext = nc.declare_dram_parameter("sums", [1, 2 * FCOLS], F32, isOutput=True)
    ft_ext = nc.declare_dram_parameter("ft", [n_chunks, FPAD, CHUNK], FP16, isOutput=True)

    ctx = contextlib.ExitStack()
    with ctx:
        ctx.enter_context(nc.allow_low_precision("fp16 feature storage by design"))
        tc = ctx.enter_context(tile.TileContext(nc))
        const = ctx.enter_context(tc.tile_pool(name="const", bufs=1))
        work = ctx.enter_context(tc.tile_pool(name="work", bufs=2))
        fpool = ctx.enter_context(tc.tile_pool(name="fpool", bufs=2))
        acc = ctx.enter_context(tc.tile_pool(name="acc", bufs=1, space="PSUM"))
        tp = ctx.enter_context(tc.tile_pool(name="tp", bufs=3, space="PSUM"))

        w32 = const.tile([128, 30], F32, tag="w32")
        ident = const.tile([128, 128], FP16, tag="ident")
        ones16 = const.tile([128, 1], FP16, tag="ones16")
        ones_bf = const.tile([128, 1], BF16, tag="ones_bf")
        nc.sync.dma_start(w32[:], w_ext[:])
        nc.sync.dma_start(ident[:], id_ext[:])
        nc.vector.memset(ones16[:], 1.0)
        nc.vector.memset(ones_bf[:], 1.0)

        sF_a = acc.tile([1, 512], F32, tag="sF_a")
        sF_b = acc.tile([1, FCOLS - 512], F32, tag="sF_b")
        sQ_a = acc.tile([1, 512], F32, tag="sQ_a")
        sQ_b = acc.tile([1, FCOLS - 512], F32, tag="sQ_b")

        for it in range(ngroups):
            c0 = it * g
            x32 = work.tile([128, g * 270], F32, tag="x32")
            nc.sync.dma_start(
                x32[:].rearrange("p (g q) -> p g q", g=g, q=270),
                x_ext[c0 * CHUNK:(c0 + g) * CHUNK, :]
                    .rearrange("(g p) q -> p g q", g=g, p=128),
            )
            xv = x32[:].rearrange("p (g f w d) -> p g f w d", g=g, f=NF, w=NW, d=ND)

            # window means (exact, f32)
            msum = work.tile([128, g * 27], F32, tag="msum")
            nc.vector.tensor_reduce(
                out=msum[:].rearrange("p (g q) -> p g q", g=g, q=27),
                in_=xv, axis=AX.X, op=ALU.add)
            m32 = work.tile([128, g * 27], F32, tag="m32")
            nc.vector.tensor_scalar_mul(m32[:], msum[:], 0.1)
            mv = m32[:].rearrange("p (g f w) -> p g f w", g=g, f=NF, w=NW)

            # centered values (f32: needed for proportional accuracy at small c)
            cc = work.tile([128, g * 270], F32, tag="cc")
            nc.gpsimd.tensor_tensor(
                out=cc[:].rearrange("p (g f w d) -> p g f w d", g=g, f=NF, w=NW, d=ND),
                in0=xv,
                in1=mv.unsqueeze(4).broadcast_to([128, g, NF, NW, ND]),
                op=ALU.subtract,
            )
            cv = cc[:].rearrange("p (g f w d) -> p g f w d", g=g, f=NF, w=NW, d=ND)

            # diag products c*c -> fp16
            cd16 = work.tile([128, g * 270], FP16, tag="cd16")
            nc.gpsimd.tensor_tensor(out=cd16[:], in0=cc[:], in1=cc[:], op=ALU.mult)

            # off-diagonal pair products -> fp16
            p16 = work.tile([128, g * NPAIR * 30], FP16, tag="p16")
            pv = p16[:].rearrange("p (g k w d) -> p g k w d", g=g, k=NPAIR, w=NW, d=ND)
            base = 0
            for i in range(NF - 1):
                nj = NF - 1 - i
                nc.vector.tensor_tensor(
                    out=pv[:, :, base:base + nj],
                    in0=cv[:, :, i:i + 1].broadcast_to([128, g, nj, NW, ND]),
                    in1=cv[:, :, i + 1:],
                    op=ALU.mult,
                )
                base += nj

            # window sums
            varS = work.tile([128, g * 27], F32, tag="varS")
            nc.vector.tensor_reduce(
                out=varS[:].rearrange("p (g q) -> p g q", g=g, q=27),
                in_=cd16[:].rearrange("p (g f w d) -> p g f w d", g=g, f=NF, w=NW, d=ND),
                axis=AX.X, op=ALU.add,
            )

            # F assembly: per-g feature block layout [128, g, 768]
            F = fpool.tile([128, g * FPAD], FP16, tag="F")
            Fg = F[:].rearrange("p (g q) -> p g q", g=g, q=FPAD)
            nc.gpsimd.memset(Fg[:, :, FCOLS:FPAD], 0.0)

            # cov stored = S (window sums of c_i c_j) straight into F
            nc.vector.tensor_reduce(
                out=Fg[:, :, 108:216].rearrange("p g (k w) -> p g k w", k=NPAIR, w=NW),
                in_=pv, axis=AX.X, op=ALU.add,
            )

            # sd = sqrt(varS/10) = K*std
            sd32 = work.tile([128, g * 27], F32, tag="sd32")
            nc.scalar.activation(out=sd32[:], in_=varS[:], func=AF.Sqrt, scale=0.1)
            nc.scalar.copy(out=Fg[:, :, 216:243], in_=sd32[:].rearrange("p (g q) -> p g q", g=g, q=27))
            nc.scalar.copy(out=Fg[:, :, 324:351], in_=m32[:].rearrange("p (g q) -> p g q", g=g, q=27))

            # corr = S / (10*sd_i*sd_j + 9e-8)
            den = work.tile([128, g * 108], F32, tag="den")
            dnv = den[:].rearrange("p (g k w) -> p g k w", g=g, k=NPAIR, w=NW)
            sdv = sd32[:].rearrange("p (g f w) -> p g f w", g=g, f=NF, w=NW)
            base = 0
            for i in range(NF - 1):
                nj = NF - 1 - i
                nc.vector.tensor_tensor(
                    out=dnv[:, :, base:base + nj],
                    in0=sdv[:, :, i:i + 1].broadcast_to([128, g, nj, NW]),
                    in1=sdv[:, :, i + 1:],
                    op=ALU.mult,
                )
                base += nj
            nc.vector.tensor_scalar(
                out=den[:], in0=den[:], scalar1=10.0, scalar2=9.0 * EPS,
                op0=ALU.mult, op1=ALU.add,
            )
            rec = work.tile([128, g * 108], F32, tag="rec")
            nc.vector.reciprocal(out=rec[:], in_=den[:])
            nc.gpsimd.tensor_tensor(
                out=Fg[:, :, 0:108],
                in0=Fg[:, :, 108:216],
                in1=rec[:].rearrange("p (g q) -> p g q", g=g, q=108),
                op=ALU.mult,
            )

            # zscore' = m / (sd + K*eps)
            zden = work.tile([128, g * 27], F32, tag="zden")
            nc.vector.tensor_scalar_add(zden[:], sd32[:], KSTD * EPS)
            zrec = work.tile([128, g * 27], F32, tag="zrec")
            nc.vector.reciprocal(out=zrec[:], in_=zden[:])
            nc.gpsimd.tensor_tensor(
                out=Fg[:, :, 243:270],
                in0=m32[:].rearrange("p (g q) -> p g q", g=g, q=27),
                in1=zrec[:].rearrange("p (g q) -> p g q", g=g, q=27),
                op=ALU.mult,
            )

            # q = x_last / x_first
            qrec = work.tile([128, g * 27], F32, tag="qrec")
            nc.vector.reciprocal(
                out=qrec[:].rearrange("p (g q) -> p g q", g=g, q=27),
                in_=xv[:, :, :, :, 0])
            nc.gpsimd.tensor_tensor(
                out=Fg[:, :, 270:297],
                in0=xv[:, :, :, :, ND - 1].rearrange("p g f w -> p g (f w)"),
                in1=qrec[:].rearrange("p (g q) -> p g q", g=g, q=27),
                op=ALU.mult,
            )

            # decay' = sum_d x*d
            dw16 = work.tile([128, g * 270], FP16, tag="dw16")
            nc.gpsimd.tensor_tensor(
                out=dw16[:].rearrange("p (g f w d) -> p g f w d", g=g, f=NF, w=NW, d=ND),
                in0=xv,
                in1=w32[:].rearrange("p (w d) -> p w d", w=NW, d=ND)
                    .unsqueeze(1).unsqueeze(1).broadcast_to([128, g, NF, NW, ND]),
                op=ALU.mult,
            )
            nc.vector.tensor_reduce(
                out=Fg[:, :, 297:324],
                in_=dw16[:].rearrange("p (g f w d) -> p g f w d", g=g, f=NF, w=NW, d=ND),
                axis=AX.X, op=ALU.add,
            )

            # window aggregates over the 351 xcat columns
            fu = Fg[:, :, 0:351].rearrange("p g (f w) -> p g f w", f=NFEAT, w=NW)
            tmpa = work.tile([128, g * NFEAT], FP16, tag="tmpa")
            tav = tmpa[:].rearrange("p (g q) -> p g q", g=g, q=NFEAT)
            nc.vector.tensor_tensor(out=tav, in0=fu[:, :, :, 0], in1=fu[:, :, :, 1], op=ALU.add)
            nc.vector.tensor_tensor(out=Fg[:, :, 351:468], in0=tav, in1=fu[:, :, :, 2], op=ALU.add)
            tmpb = work.tile([128, g * NFEAT], FP16, tag="tmpb")
            tbv = tmpb[:].rearrange("p (g q) -> p g q", g=g, q=NFEAT)
            nc.vector.tensor_tensor(out=tbv, in0=fu[:, :, :, 0], in1=fu[:, :, :, 1], op=ALU.max)
            nc.vector.tensor_tensor(out=Fg[:, :, 468:585], in0=tbv, in1=fu[:, :, :, 2], op=ALU.max)
            tmpc = work.tile([128, g * NFEAT], FP16, tag="tmpc")
            tcv = tmpc[:].rearrange("p (g q) -> p g q", g=g, q=NFEAT)
            nc.vector.tensor_tensor(out=tcv, in0=fu[:, :, :, 0], in1=fu[:, :, :, 1], op=ALU.min)
            nc.vector.tensor_tensor(out=Fg[:, :, 585:702], in0=tcv, in1=fu[:, :, :, 2], op=ALU.min)

            # stats: column sums of F and F^2 (F2 in bf16 to keep 16-bit matmuls)
            F2 = fpool.tile([128, g * FCOLS], BF16, tag="F2")
            F2g = F2[:].rearrange("p (g q) -> p g q", g=g, q=FCOLS)
            nc.scalar.activation(out=F2[:].rearrange("p (g q) -> p g q", g=g, q=FCOLS),
                                 in_=Fg[:, :, 0:FCOLS], func=AF.Square)
            for gg in range(g):
                first = (it == 0 and gg == 0)
                last = (it == ngroups - 1 and gg == g - 1)
                nc.tensor.matmul(out=sF_a[:], lhsT=ones16[:], rhs=Fg[:, gg, 0:512],
                                 start=first, stop=last)
                nc.tensor.matmul(out=sF_b[:], lhsT=ones16[:], rhs=Fg[:, gg, 512:FCOLS],
                                 start=first, stop=last)
            for gg in range(g):
                first = (it == 0 and gg == 0)
                last = (it == ngroups - 1 and gg == g - 1)
                nc.tensor.matmul(out=sQ_a[:], lhsT=ones_bf[:], rhs=F2g[:, gg, 0:512],
                                 start=first, stop=last)
                nc.tensor.matmul(out=sQ_b[:], lhsT=ones_bf[:], rhs=F2g[:, gg, 512:FCOLS],
                                 start=first, stop=last)

            # transpose F -> ft
            ftile = fpool.tile([128, g * 6 * 128], FP16, tag="ftile")
            fts = ftile[:].rearrange("p (g k r) -> p g k r", g=g, k=6, r=128)
            for gg in range(g):
                for p in range(6):
                    pt = tp.tile([128, 128], FP16, tag="pt")
                    nc.tensor.transpose(out=pt[:], in_=Fg[:, gg, p * 128:(p + 1) * 128],
                                        identity=ident[:])
                    if p % 2 == 0:
                        nc.scalar.copy(out=fts[:, gg, p, :], in_=pt[:])
                    else:
                        nc.vector.tensor_copy(fts[:, gg, p, :], pt[:])
            nc.sync.dma_start(
                ft_ext[c0:c0 + g].rearrange("g (k p) r -> p g k r", k=6, p=128),
                fts,
            )

        sums_sb = const.tile([1, 2 * FCOLS], F32, tag="sums_sb")
        nc.scalar.copy(out=sums_sb[:, 0:512], in_=sF_a[:])
        nc.scalar.copy(out=sums_sb[:, 512:FCOLS], in_=sF_b[:])
        nc.scalar.copy(out=sums_sb[:, FCOLS:FCOLS + 512], in_=sQ_a[:])
        nc.scalar.copy(out=sums_sb[:, FCOLS + 512:2 * FCOLS], in_=sQ_b[:])
        nc.sync.dma_start(sums_ext[:], sums_sb[:])

    split_multi_waits(nc)
    return nc


# ---------------------------------------------------------------------------
def build_neff_b(n_chunks=NCHUNKS, g=GB):
    assert n_chunks % g == 0
    ngroups = n_chunks // g
    nc = bass.Bass()
    ft_ext = nc.declare_dram_parameter("ft", [n_chunks, FPAD, CHUNK], FP16, isOutput=False)
    w1_ext = nc.declare_dram_parameter("w1t", [FPAD, 30], FP16, isOutput=False)
    b1_ext = nc.declare_dram_parameter("b1p", [30, 1], F32, isOutput=False)
    w2_ext = nc.declare_dram_parameter("w2p", [30, 1], FP16, isOutput=False)
    bo_ext = nc.declare_dram_parameter("boutp", [1, 1], F32, isOutput=False)
    out_ext = nc.declare_dram_parameter("out", [1, n_chunks * CHUNK], F32, isOutput=True)

    ctx = contextlib.ExitStack()
    with ctx:
        ctx.enter_context(nc.allow_low_precision("fp16 mlp by design"))
        tc = ctx.enter_context(tile.TileContext(nc))
        const = ctx.enter_context(tc.tile_pool(name="const", bufs=1))
        work = ctx.enter_context(tc.tile_pool(name="work", bufs=3))
        ps = ctx.enter_context(tc.tile_pool(name="ps", bufs=2, space="PSUM"))

        w1b = const.tile([128, 6 * 30], FP16, tag="w1b")
        nc.sync.dma_start(
            w1b[:].rearrange("p (k m) -> p k m", k=6, m=30),
            w1_ext[:].rearrange("(k p) m -> p k m", k=6, p=128),
        )
        b1b = const.tile([30, 1], F32, tag="b1b")
        nc.sync.dma_start(b1b[:], b1_ext[:])
        w2b = const.tile([30, 1], FP16, tag="w2b")
        nc.sync.dma_start(w2b[:], w2_ext[:])
        bob = const.tile([1, 1], F32, tag="bob")
        nc.sync.dma_start(bob[:], bo_ext[:])
        out_sb = const.tile([1, n_chunks * CHUNK], F32, tag="out_sb")

        for it in range(ngroups):
            c0 = it * g
            ftb = work.tile([128, g * 6 * 128], FP16, tag="ftb")
            nc.sync.dma_start(
                ftb[:].rearrange("p (g k r) -> p g k r", g=g, k=6, r=128),
                ft_ext[c0:c0 + g].rearrange("g (k p) r -> p g k r", k=6, p=128),
            )
            fbv = ftb[:].rearrange("p (g k r) -> p g k r", g=g, k=6, r=128)
            h_ps = ps.tile([30, g * 128], F32, tag="h_ps")
            for p in range(6):
                nc.tensor.matmul(
                    out=h_ps[:].rearrange("p (g r) -> p g r", g=g, r=128),
                    lhsT=w1b[:, p * 30:(p + 1) * 30],
                    rhs=fbv[:, :, p, :],
                    start=(p == 0), stop=(p == 5),
                )
            h16 = work.tile([30, g * 128], FP16, tag="h16")
            nc.scalar.activation(out=h16[:], in_=h_ps[:], func=AF.Relu,
                                 bias=b1b[:], scale=1.0)
            o_ps = ps.tile([1, g * 128], F32, tag="o_ps")
            nc.tensor.matmul(out=o_ps[:], lhsT=w2b[:], rhs=h16[:],
                             start=True, stop=True)
            nc.scalar.activation(out=out_sb[:, c0 * CHUNK:(c0 + g) * CHUNK],
                                 in_=o_ps[:], func=AF.Identity,
                                 bias=bob[:], scale=1.0)

        nc.sync.dma_start(out_ext[:], out_sb[:])

    split_multi_waits(nc)
    return nc


# ---------------------------------------------------------------------------
def fold_weights(sums, n_rows, gamma, beta, W1, b1, W2, b2, w_out, b_out):
    """sums: [2, 702] float64 global column sums / sumsqs of the stored F."""
    s1, s2 = sums[0], sums[1]
    alpha = np.zeros(FCOLS)
    bet = np.zeros(FCOLS)
    for gi, sz in enumerate(GROUP_SIZES):
        f0 = sum(GROUP_SIZES[:gi])
        cols = slice(f0 * 3, (f0 + sz) * 3)
        cnt = n_rows * sz * 3
        mean = s1[cols].sum() / cnt
        var = s2[cols].sum() / cnt - mean ** 2
        a = gamma / np.sqrt(var + EPS_BN / S_T[gi] ** 2)
        alpha[cols] = a
        bet[cols] = beta - a * mean
    a_f = alpha[np.arange(NFEAT) * 3]
    b_f = bet[np.arange(NFEAT) * 3]
    for blk, scale in ((0, 1.0 / 3.0), (1, 1.0), (2, 1.0)):
        cols = slice(351 + blk * NFEAT, 351 + (blk + 1) * NFEAT)
        e1 = s1[cols] / n_rows
        e2 = s2[cols] / n_rows
        mean_all = (a_f * scale * e1 + b_f).mean()
        ex2_all = ((a_f * scale) ** 2 * e2 + 2 * a_f * scale * b_f * e1 + b_f ** 2).mean()
        var_all = ex2_all - mean_all ** 2
        a2 = gamma / np.sqrt(var_all + EPS_BN)
        b2_ = beta - a2 * mean_all
        alpha[cols] = a2 * a_f * scale
        bet[cols] = a2 * b_f + b2_
    W1p = W1 * alpha[None, :]
    b1p = b1 + W1 @ bet
    w1t = np.zeros((FPAD, 30), np.float16)
    w1t[:FCOLS, :] = W1p.T.astype(np.float16)
    b1p = b1p.reshape(30, 1).astype(np.float32)
    w2p = (W2[0] * float(w_out[0])).reshape(30, 1).astype(np.float16)
    boutp = np.array([[float(b2[0]) * float(w_out[0]) + float(b_out[0])]], np.float32)
    return w1t, b1p, w2p, boutp


# ---------------------------------------------------------------------------
_CACHE = {}


def _get_neffs():
    if "a" not in _CACHE:
        _CACHE["a"] = build_neff_a()
        _CACHE["b"] = build_neff_b()
    return _CACHE["a"], _CACHE["b"]


def _wday_tile():
    w = np.tile(np.arange(1, ND + 1, dtype=np.float32), NW)
    return np.tile(w[None, :], (128, 1)).copy()


def kernel(xb, gamma, beta, W1, b1, W2, b2, w_out, b_out):
    xb = np.ascontiguousarray(np.asarray(xb, dtype=np.float32))
    x_flat = xb.reshape(B_TOTAL, 270)
    shards = [np.ascontiguousarray(x_flat[i * ROWS:(i + 1) * ROWS]) for i in range(NCORES)]

    nc_a, nc_b = _get_neffs()
    wday = _wday_tile()
    ident = np.eye(128, dtype=np.float16)

    in_maps_a = [{"x": shards[i], "wday": wday, "ident": ident} for i in range(NCORES)]
    res_a = run_bass_kernel_spmd(nc_a, in_maps_a, core_ids=list(range(NCORES)))

    sums = np.zeros((2, FCOLS), np.float64)
    fts = []
    for i in range(NCORES):
        s = res_a.results[i]["sums"].reshape(2, FCOLS).astype(np.float64)
        sums += s
        fts.append(res_a.results[i]["ft"])

    w1t, b1p, w2p, boutp = fold_weights(
        sums, B_TOTAL,
        float(np.asarray(gamma).reshape(-1)[0]), float(np.asarray(beta).reshape(-1)[0]),
        np.asarray(W1, np.float64), np.asarray(b1, np.float64),
        np.asarray(W2, np.float64), np.asarray(b2, np.float64),
        np.asarray(w_out, np.float64), np.asarray(b_out, np.float64),
    )

    in_maps_b = [
        {"ft": fts[i], "w1t": w1t, "b1p": b1p, "w2p": w2p, "boutp": boutp}
        for i in range(NCORES)
    ]
    res_b = run_bass_kernel_spmd(nc_b, in_maps_b, core_ids=list(range(NCORES)))

    out = np.concatenate([res_b.results[i]["out"].reshape(-1) for i in range(NCORES)])
    return out.astype(np.float32)


# revision 8
# speedup vs baseline: 1.2005x; 1.0473x over previous
"""AlphaNet_v1 Trainium2 kernel — 8-core data-parallel Bass implementation.

Structure (per core, shard = 16384 rows of xb):
  NEFF-A: one pass over the shard, G=4 chunks of 128 rows per iteration.
      Compute the raw (scale-folded) AlphaNet features F [128, G*702] fp16,
      accumulate per-column sum / sum-of-squares via PE column-sum matmuls,
      PE-transpose F and spill F^T to DRAM.
  Host: reduce the column sums across cores (numpy), build the BatchNorm
      affine per feature column, fold into the MLP weights (BN is affine
      a*x+b per tensor; max/min over windows commute since a>0 for gamma=1;
      stored-feature scale factors are absorbed exactly by scaling BN's
      epsilon per group).
  NEFF-B: stream F^T back, accumulated matmuls against folded W1', fused
      bias+relu on ScalarE, final matmul against W2', output.

Stored feature scales (ref = s * stored): corr 1, cov(S = window sum of
c_i*c_j) 1/9, sd(=sqrt(S_ii/10)) sqrt(10/9), zs(=m/(sd+K*eps)) K=sqrt(9/10),
q(=xl/xf; the -1 shift is BN-invariant) 1, decay(=sum x*d) 1/55, mean 1;
ubar stores the window SUM (handled in fold).
"""

import contextlib
import numpy as np

import bass_rust
import concourse.bass as bass
import concourse.mybir as mybir
import concourse.tile as tile
from concourse.bass_utils import run_bass_kernel_spmd

F32 = mybir.dt.float32
FP16 = mybir.dt.float16
BF16 = mybir.dt.bfloat16
ALU = mybir.AluOpType
AF = mybir.ActivationFunctionType
AX = mybir.AxisListType

NCORES = 8
B_TOTAL = 131072
ROWS = B_TOTAL // NCORES          # 16384 rows per core
CHUNK = 128
NCHUNKS = ROWS // CHUNK           # 128
GA = 4                            # chunks per iteration, NEFF-A
GB = 4                            # chunks per iteration, NEFF-B
NF, NW, ND = 9, 3, 10
NPAIR = 36
NFEAT = 117
FCOLS = 702
FPAD = 768
EPS_BN, EPS = 1e-5, 1e-8
KSTD = float(np.sqrt(0.9))

I_IDX, J_IDX = np.triu_indices(NF, k=1)
GROUP_SIZES = [36, 36, 9, 9, 9, 9, 9]
# ref = s * stored, per xcat group [corr, cov, sd, zs, q, decay, m]
S_T = [1.0, 1.0 / 9.0, float(np.sqrt(10.0 / 9.0)), KSTD, 1.0, 1.0 / 55.0, 1.0]


# ---------------------------------------------------------------------------
# toolchain workaround: this walrus build allows only ONE semaphore wait per
# instruction; Tile sometimes attaches more. Hoist extras onto standalone
# Drain instructions inserted before the offender on the same engine.
_wsplit_n = [0]


def split_multi_waits(nc):
    for fn in nc.m.functions:
        for bb in fn.blocks:
            new_list = []
            for ins in bb.instructions:
                si = ins.sync_info
                waits = list(si.on_wait) if (si is not None and si.on_wait) else []
                if len(waits) > 1:
                    for w in waits[:-1]:
                        _wsplit_n[0] += 1
                        d = bass_rust.InstDrain(
                            name=f"wsplit-{_wsplit_n[0]}", ins=[], outs=[]
                        )
                        d.engine = ins.engine
                        d.sync_info = mybir.SyncInfo(on_wait=[w], on_update=[])
                        new_list.append(d)
                    si.on_wait = [waits[-1]]
                new_list.append(ins)
            bb.instructions[:] = new_list


# ---------------------------------------------------------------------------
def build_neff_a(n_chunks=NCHUNKS, g=GA):
    assert n_chunks % g == 0
    ngroups = n_chunks // g
    nc = bass.Bass()
    x_ext = nc.declare_dram_parameter("x", [n_chunks * CHUNK, 270], F32, isOutput=False)
    w_ext = nc.declare_dram_parameter("wday", [128, 30], F32, isOutput=False)
    id_ext = nc.declare_dram_parameter("ident", [128, 128], FP16, isOutput=False)
    sums_ext = nc.declare_dram_parameter("sums", [1, 2 * FCOLS], F32, isOutput=True)
    ft_ext = nc.declare_dram_parameter("ft", [n_chunks, FPAD, CHUNK], FP16, isOutput=True)

    ctx = contextlib.ExitStack()
    with ctx:
        ctx.enter_context(nc.allow_low_precision("fp16 feature storage by design"))
        tc = ctx.enter_context(tile.TileContext(nc))
        const = ctx.enter_context(tc.tile_pool(name="const", bufs=1))
        work = ctx.enter_context(tc.tile_pool(name="work", bufs=3))
        fpool = ctx.enter_context(tc.tile_pool(name="fpool", bufs=2))
        acc = ctx.enter_context(tc.tile_pool(name="acc", bufs=1, space="PSUM"))
        tp = ctx.enter_context(tc.tile_pool(name="tp", bufs=3, space="PSUM"))

        w32 = const.tile([128, 30], F32, tag="w32")
        ident = const.tile([128, 128], FP16, tag="ident")
        ones16 = const.tile([128, 1], FP16, tag="ones16")
        ones_bf = const.tile([128, 1], BF16, tag="ones_bf")
        nc.sync.dma_start(w32[:], w_ext[:])
        nc.sync.dma_start(ident[:], id_ext[:])
        nc.vector.memset(ones16[:], 1.0)
        nc.vector.memset(ones_bf[:], 1.0)

        sF_a = acc.tile([1, 512], F32, tag="sF_a")
        sF_b = acc.tile([1, FCOLS - 512], F32, tag="sF_b")
        sQ_a = acc.tile([1, 512], F32, tag="sQ_a")
        sQ_b = acc.tile([1, FCOLS - 512], F32, tag="sQ_b")

        for it in range(ngroups):
            c0 = it * g
            x32 = work.tile([128, g * 270], F32, tag="x32")
            nc.sync.dma_start(
                x32[:].rearrange("p (g q) -> p g q", g=g, q=270),
                x_ext[c0 * CHUNK:(c0 + g) * CHUNK, :]
                    .rearrange("(g p) q -> p g q", g=g, p=128),
            )
            xv = x32[:].rearrange("p (g f w d) -> p g f w d", g=g, f=NF, w=NW, d=ND)

            # window means (exact, f32)
            msum = work.tile([128, g * 27], F32, tag="msum")
            nc.vector.tensor_reduce(
                out=msum[:].rearrange("p (g q) -> p g q", g=g, q=27),
                in_=xv, axis=AX.X, op=ALU.add)
            m32 = work.tile([128, g * 27], F32, tag="m32")
            nc.vector.tensor_scalar_mul(m32[:], msum[:], 0.1)
            mv = m32[:].rearrange("p (g f w) -> p g f w", g=g, f=NF, w=NW)

            # centered values (f32: proportional accuracy at small c)
            cc = work.tile([128, g * 270], F32, tag="cc")
            nc.gpsimd.tensor_tensor(
                out=cc[:].rearrange("p (g f w d) -> p g f w d", g=g, f=NF, w=NW, d=ND),
                in0=xv,
                in1=mv.unsqueeze(4).broadcast_to([128, g, NF, NW, ND]),
                op=ALU.subtract,
            )
            cv = cc[:].rearrange("p (g f w d) -> p g f w d", g=g, f=NF, w=NW, d=ND)

            # diag products c*c -> fp16, then window var sum
            cd16 = work.tile([128, g * 270], FP16, tag="cd16")
            nc.gpsimd.tensor_tensor(out=cd16[:], in0=cc[:], in1=cc[:], op=ALU.mult)
            varS = work.tile([128, g * 27], F32, tag="varS")
            nc.vector.tensor_reduce(
                out=varS[:].rearrange("p (g q) -> p g q", g=g, q=27),
                in_=cd16[:].rearrange("p (g f w d) -> p g f w d", g=g, f=NF, w=NW, d=ND),
                axis=AX.X, op=ALU.add,
            )

            # off-diagonal pair products -> fp16
            p16 = work.tile([128, g * NPAIR * 30], FP16, tag="p16")
            pv = p16[:].rearrange("p (g k w d) -> p g k w d", g=g, k=NPAIR, w=NW, d=ND)
            base = 0
            for i in range(NF - 1):
                nj = NF - 1 - i
                nc.vector.tensor_tensor(
                    out=pv[:, :, base:base + nj],
                    in0=cv[:, :, i + 1:],
                    in1=cv[:, :, i:i + 1].broadcast_to([128, g, nj, NW, ND]),
                    op=ALU.mult,
                )
                base += nj


            # F assembly: per-g feature block layout [128, g, 768]
            F = fpool.tile([128, g * FPAD], FP16, tag="F")
            Fg = F[:].rearrange("p (g q) -> p g q", g=g, q=FPAD)
            nc.gpsimd.memset(Fg[:, :, FCOLS:FPAD], 0.0)

            # cov stored = S (window sums of c_i c_j) straight into F
            nc.vector.tensor_reduce(
                out=Fg[:, :, 108:216].rearrange("p g (k w) -> p g k w", k=NPAIR, w=NW),
                in_=pv, axis=AX.X, op=ALU.add,
            )

            # sd = sqrt(varS/10) = K*std
            sd32 = work.tile([128, g * 27], F32, tag="sd32")
            nc.scalar.activation(out=sd32[:], in_=varS[:], func=AF.Sqrt, scale=0.1)
            nc.scalar.copy(out=Fg[:, :, 216:243], in_=sd32[:].rearrange("p (g q) -> p g q", g=g, q=27))
            nc.scalar.copy(out=Fg[:, :, 324:351], in_=m32[:].rearrange("p (g q) -> p g q", g=g, q=27))

            # corr = S / (10*sd_i*sd_j + 9e-8)
            den = work.tile([128, g * 108], F32, tag="den")
            dnv = den[:].rearrange("p (g k w) -> p g k w", g=g, k=NPAIR, w=NW)
            sdv = sd32[:].rearrange("p (g f w) -> p g f w", g=g, f=NF, w=NW)
            base = 0
            for i in range(NF - 1):
                nj = NF - 1 - i
                nc.vector.tensor_tensor(
                    out=dnv[:, :, base:base + nj],
                    in0=sdv[:, :, i + 1:],
                    in1=sdv[:, :, i:i + 1].broadcast_to([128, g, nj, NW]),
                    op=ALU.mult,
                )
                base += nj
            nc.vector.tensor_scalar(
                out=den[:], in0=den[:], scalar1=10.0, scalar2=9.0 * EPS,
                op0=ALU.mult, op1=ALU.add,
            )
            rec = work.tile([128, g * 108], F32, tag="rec")
            nc.vector.reciprocal(out=rec[:], in_=den[:])
            nc.gpsimd.tensor_tensor(
                out=Fg[:, :, 0:108],
                in0=Fg[:, :, 108:216],
                in1=rec[:].rearrange("p (g q) -> p g q", g=g, q=108),
                op=ALU.mult,
            )

            # zscore' = m / (sd + K*eps)
            zden = work.tile([128, g * 27], F32, tag="zden")
            nc.vector.tensor_scalar_add(zden[:], sd32[:], KSTD * EPS)
            zrec = work.tile([128, g * 27], F32, tag="zrec")
            nc.vector.reciprocal(out=zrec[:], in_=zden[:])
            nc.gpsimd.tensor_tensor(
                out=Fg[:, :, 243:270],
                in0=m32[:].rearrange("p (g q) -> p g q", g=g, q=27),
                in1=zrec[:].rearrange("p (g q) -> p g q", g=g, q=27),
                op=ALU.mult,
            )

            # q = x_last / x_first
            qrec = work.tile([128, g * 27], F32, tag="qrec")
            nc.vector.reciprocal(
                out=qrec[:].rearrange("p (g q) -> p g q", g=g, q=27),
                in_=xv[:, :, :, :, 0])
            nc.gpsimd.tensor_tensor(
                out=Fg[:, :, 270:297],
                in0=xv[:, :, :, :, ND - 1].rearrange("p g f w -> p g (f w)"),
                in1=qrec[:].rearrange("p (g q) -> p g q", g=g, q=27),
                op=ALU.mult,
            )

            # decay' = sum_d x*d
            dw16 = work.tile([128, g * 270], FP16, tag="dw16")
            nc.gpsimd.tensor_tensor(
                out=dw16[:].rearrange("p (g f w d) -> p g f w d", g=g, f=NF, w=NW, d=ND),
                in0=xv,
                in1=w32[:].rearrange("p (w d) -> p w d", w=NW, d=ND)
                    .unsqueeze(1).unsqueeze(1).broadcast_to([128, g, NF, NW, ND]),
                op=ALU.mult,
            )
            dwt = work.tile([128, g * 135], FP16, tag="dwt")
            dwv = dw16[:].rearrange("p (g f w d) -> p g f w d", g=g, f=NF, w=NW, d=ND)
            nc.gpsimd.tensor_tensor(
                out=dwt[:].rearrange("p (g f w d) -> p g f w d", g=g, f=NF, w=NW, d=5),
                in0=dwv[:, :, :, :, 0:5], in1=dwv[:, :, :, :, 5:10], op=ALU.add)
            nc.vector.tensor_reduce(
                out=Fg[:, :, 297:324],
                in_=dwt[:].rearrange("p (g f w d) -> p g f w d", g=g, f=NF, w=NW, d=5),
                axis=AX.X, op=ALU.add,
            )

            # window aggregates over the 351 xcat columns
            fu = Fg[:, :, 0:351].rearrange("p g (f w) -> p g f w", f=NFEAT, w=NW)
            tmpa = work.tile([128, g * NFEAT], FP16, tag="tmpa")
            tav = tmpa[:].rearrange("p (g q) -> p g q", g=g, q=NFEAT)
            nc.vector.tensor_tensor(out=tav, in0=fu[:, :, :, 0], in1=fu[:, :, :, 1], op=ALU.add)
            nc.vector.tensor_tensor(out=Fg[:, :, 351:468], in0=tav, in1=fu[:, :, :, 2], op=ALU.add)
            tmpb = work.tile([128, g * NFEAT], FP16, tag="tmpb")
            tbv = tmpb[:].rearrange("p (g q) -> p g q", g=g, q=NFEAT)
            nc.vector.tensor_tensor(out=tbv, in0=fu[:, :, :, 0], in1=fu[:, :, :, 1], op=ALU.max)
            nc.vector.tensor_tensor(out=Fg[:, :, 468:585], in0=tbv, in1=fu[:, :, :, 2], op=ALU.max)
            tmpc = work.tile([128, g * NFEAT], FP16, tag="tmpc")
            tcv = tmpc[:].rearrange("p (g q) -> p g q", g=g, q=NFEAT)
            nc.vector.tensor_tensor(out=tcv, in0=fu[:, :, :, 0], in1=fu[:, :, :, 1], op=ALU.min)
            nc.vector.tensor_tensor(out=Fg[:, :, 585:702], in0=tcv, in1=fu[:, :, :, 2], op=ALU.min)

            # stats: column sums of F and F^2 (F2 in bf16 to keep 16-bit matmuls)
            F2 = fpool.tile([128, g * FCOLS], BF16, tag="F2")
            F2g = F2[:].rearrange("p (g q) -> p g q", g=g, q=FCOLS)
            nc.scalar.activation(out=F2[:].rearrange("p (g q) -> p g q", g=g, q=FCOLS),
                                 in_=Fg[:, :, 0:FCOLS], func=AF.Square)
            for gg in range(g):
                first = (it == 0 and gg == 0)
                last = (it == ngroups - 1 and gg == g - 1)
                nc.tensor.matmul(out=sF_a[:], lhsT=ones16[:], rhs=Fg[:, gg, 0:512],
                                 start=first, stop=last)
                nc.tensor.matmul(out=sF_b[:], lhsT=ones16[:], rhs=Fg[:, gg, 512:FCOLS],
                                 start=first, stop=last)
            for gg in range(g):
                first = (it == 0 and gg == 0)
                last = (it == ngroups - 1 and gg == g - 1)
                nc.tensor.matmul(out=sQ_a[:], lhsT=ones_bf[:], rhs=F2g[:, gg, 0:512],
                                 start=first, stop=last)
                nc.tensor.matmul(out=sQ_b[:], lhsT=ones_bf[:], rhs=F2g[:, gg, 512:FCOLS],
                                 start=first, stop=last)

            # transpose F -> ft
            ftile = fpool.tile([128, g * 6 * 128], FP16, tag="ftile")
            fts = ftile[:].rearrange("p (g k r) -> p g k r", g=g, k=6, r=128)
            for gg in range(g):
                for p in range(6):
                    pt = tp.tile([128, 128], FP16, tag="pt")
                    nc.tensor.transpose(out=pt[:], in_=Fg[:, gg, p * 128:(p + 1) * 128],
                                        identity=ident[:])
                    nc.scalar.copy(out=fts[:, gg, p, :], in_=pt[:])
            nc.sync.dma_start(
                ft_ext[c0:c0 + g].rearrange("g (k p) r -> p g k r", k=6, p=128),
                fts,
            )

        sums_sb = const.tile([1, 2 * FCOLS], F32, tag="sums_sb")
        nc.scalar.copy(out=sums_sb[:, 0:512], in_=sF_a[:])
        nc.scalar.copy(out=sums_sb[:, 512:FCOLS], in_=sF_b[:])
        nc.scalar.copy(out=sums_sb[:, FCOLS:FCOLS + 512], in_=sQ_a[:])
        nc.scalar.copy(out=sums_sb[:, FCOLS + 512:2 * FCOLS], in_=sQ_b[:])
        nc.sync.dma_start(sums_ext[:], sums_sb[:])

    split_multi_waits(nc)
    return nc


# ---------------------------------------------------------------------------
def build_neff_b(n_chunks=NCHUNKS, g=GB):
    assert n_chunks % g == 0
    ngroups = n_chunks // g
    nc = bass.Bass()
    ft_ext = nc.declare_dram_parameter("ft", [n_chunks, FPAD, CHUNK], FP16, isOutput=False)
    w1_ext = nc.declare_dram_parameter("w1t", [FPAD, 30], FP16, isOutput=False)
    b1_ext = nc.declare_dram_parameter("b1p", [30, 1], F32, isOutput=False)
    w2_ext = nc.declare_dram_parameter("w2p", [30, 1], FP16, isOutput=False)
    bo_ext = nc.declare_dram_parameter("boutp", [1, 1], F32, isOutput=False)
    out_ext = nc.declare_dram_parameter("out", [1, n_chunks * CHUNK], F32, isOutput=True)

    ctx = contextlib.ExitStack()
    with ctx:
        ctx.enter_context(nc.allow_low_precision("fp16 mlp by design"))
        tc = ctx.enter_context(tile.TileContext(nc))
        const = ctx.enter_context(tc.tile_pool(name="const", bufs=1))
        work = ctx.enter_context(tc.tile_pool(name="work", bufs=3))
        ps = ctx.enter_context(tc.tile_pool(name="ps", bufs=2, space="PSUM"))

        w1b = const.tile([128, 6 * 30], FP16, tag="w1b")
        nc.sync.dma_start(
            w1b[:].rearrange("p (k m) -> p k m", k=6, m=30),
            w1_ext[:].rearrange("(k p) m -> p k m", k=6, p=128),
        )
        b1b = const.tile([30, 1], F32, tag="b1b")
        nc.sync.dma_start(b1b[:], b1_ext[:])
        w2b = const.tile([30, 1], FP16, tag="w2b")
        nc.sync.dma_start(w2b[:], w2_ext[:])
        bob = const.tile([1, 1], F32, tag="bob")
        nc.sync.dma_start(bob[:], bo_ext[:])
        out_sb = const.tile([1, n_chunks * CHUNK], F32, tag="out_sb")

        for it in range(ngroups):
            c0 = it * g
            ftb = work.tile([128, g * 6 * 128], FP16, tag="ftb")
            nc.sync.dma_start(
                ftb[:].rearrange("p (g k r) -> p g k r", g=g, k=6, r=128),
                ft_ext[c0:c0 + g].rearrange("g (k p) r -> p g k r", k=6, p=128),
            )
            fbv = ftb[:].rearrange("p (g k r) -> p g k r", g=g, k=6, r=128)
            h_ps = ps.tile([30, g * 128], F32, tag="h_ps")
            for p in range(6):
                nc.tensor.matmul(
                    out=h_ps[:].rearrange("p (g r) -> p g r", g=g, r=128),
                    lhsT=w1b[:, p * 30:(p + 1) * 30],
                    rhs=fbv[:, :, p, :],
                    start=(p == 0), stop=(p == 5),
                )
            h16 = work.tile([30, g * 128], FP16, tag="h16")
            nc.scalar.activation(out=h16[:], in_=h_ps[:], func=AF.Relu,
                                 bias=b1b[:], scale=1.0)
            o_ps = ps.tile([1, g * 128], F32, tag="o_ps")
            nc.tensor.matmul(out=o_ps[:], lhsT=w2b[:], rhs=h16[:],
                             start=True, stop=True)
            nc.scalar.activation(out=out_sb[:, c0 * CHUNK:(c0 + g) * CHUNK],
                                 in_=o_ps[:], func=AF.Identity,
                                 bias=bob[:], scale=1.0)

        nc.sync.dma_start(out_ext[:], out_sb[:])

    split_multi_waits(nc)
    return nc


# ---------------------------------------------------------------------------
def fold_weights(sums, n_rows, gamma, beta, W1, b1, W2, b2, w_out, b_out):
    """sums: [2, 702] float64 global column sums / sumsqs of the stored F."""
    s1, s2 = sums[0], sums[1]
    alpha = np.zeros(FCOLS)
    bet = np.zeros(FCOLS)
    for gi, sz in enumerate(GROUP_SIZES):
        f0 = sum(GROUP_SIZES[:gi])
        cols = slice(f0 * 3, (f0 + sz) * 3)
        cnt = n_rows * sz * 3
        mean = s1[cols].sum() / cnt
        var = s2[cols].sum() / cnt - mean ** 2
        a = gamma / np.sqrt(var + EPS_BN / S_T[gi] ** 2)
        alpha[cols] = a
        bet[cols] = beta - a * mean
    a_f = alpha[np.arange(NFEAT) * 3]
    b_f = bet[np.arange(NFEAT) * 3]
    for blk, scale in ((0, 1.0 / 3.0), (1, 1.0), (2, 1.0)):
        cols = slice(351 + blk * NFEAT, 351 + (blk + 1) * NFEAT)
        e1 = s1[cols] / n_rows
        e2 = s2[cols] / n_rows
        mean_all = (a_f * scale * e1 + b_f).mean()
        ex2_all = ((a_f * scale) ** 2 * e2 + 2 * a_f * scale * b_f * e1 + b_f ** 2).mean()
        var_all = ex2_all - mean_all ** 2
        a2 = gamma / np.sqrt(var_all + EPS_BN)
        b2_ = beta - a2 * mean_all
        alpha[cols] = a2 * a_f * scale
        bet[cols] = a2 * b_f + b2_
    W1p = W1 * alpha[None, :]
    b1p = b1 + W1 @ bet
    w1t = np.zeros((FPAD, 30), np.float16)
    w1t[:FCOLS, :] = W1p.T.astype(np.float16)
    b1p = b1p.reshape(30, 1).astype(np.float32)
    w2p = (W2[0] * float(w_out[0])).reshape(30, 1).astype(np.float16)
    boutp = np.array([[float(b2[0]) * float(w_out[0]) + float(b_out[0])]], np.float32)
    return w1t, b1p, w2p, boutp


# ---------------------------------------------------------------------------
_CACHE = {}


def _get_neffs():
    if "a" not in _CACHE:
        _CACHE["a"] = build_neff_a()
        _CACHE["b"] = build_neff_b()
    return _CACHE["a"], _CACHE["b"]


def _wday_tile():
    w = np.tile(np.arange(1, ND + 1, dtype=np.float32), NW)
    return np.tile(w[None, :], (128, 1)).copy()


def kernel(xb, gamma, beta, W1, b1, W2, b2, w_out, b_out):
    xb = np.ascontiguousarray(np.asarray(xb, dtype=np.float32))
    x_flat = xb.reshape(B_TOTAL, 270)
    shards = [np.ascontiguousarray(x_flat[i * ROWS:(i + 1) * ROWS]) for i in range(NCORES)]

    nc_a, nc_b = _get_neffs()
    wday = _wday_tile()
    ident = np.eye(128, dtype=np.float16)

    in_maps_a = [{"x": shards[i], "wday": wday, "ident": ident} for i in range(NCORES)]
    res_a = run_bass_kernel_spmd(nc_a, in_maps_a, core_ids=list(range(NCORES)))

    sums = np.zeros((2, FCOLS), np.float64)
    fts = []
    for i in range(NCORES):
        s = res_a.results[i]["sums"].reshape(2, FCOLS).astype(np.float64)
        sums += s
        fts.append(res_a.results[i]["ft"])

    w1t, b1p, w2p, boutp = fold_weights(
        sums, B_TOTAL,
        float(np.asarray(gamma).reshape(-1)[0]), float(np.asarray(beta).reshape(-1)[0]),
        np.asarray(W1, np.float64), np.asarray(b1, np.float64),
        np.asarray(W2, np.float64), np.asarray(b2, np.float64),
        np.asarray(w_out, np.float64), np.asarray(b_out, np.float64),
    )

    in_maps_b = [
        {"ft": fts[i], "w1t": w1t, "b1p": b1p, "w2p": w2p, "boutp": boutp}
        for i in range(NCORES)
    ]
    res_b = run_bass_kernel_spmd(nc_b, in_maps_b, core_ids=list(range(NCORES)))

    out = np.concatenate([res_b.results[i]["out"].reshape(-1) for i in range(NCORES)])
    return out.astype(np.float32)
